# revision 2
# baseline (speedup 1.0000x reference)
"""Cross-attention (B=16, S=2048, D=1024, fp32) on 8 TRN2 NeuronCores.

Sharding: data-parallel over batch (2 batches per core), projection weights
replicated. Inputs are pre-transposed on host to feature-major [B, D, S] so
all device-side matmuls keep the contraction dim on partitions with zero
on-device input transposes.

Per core, per batch:
  stage A: QT[f,s] = (Wq^T x^T + bq)      -> spilled to DRAM (f-major)
           KT[f,s] = (Wk^T y^T + bk)      -> SBUF resident   (f-major)
           V [s,f] = (y^T^T Wv + bv)      -> SBUF resident   (seq-major)
  stage B: per 128-row q-tile:
           logits = QT^T KT  (PSUM, 4x512 banks)
           row max (DVE), exp((l-m)/sqrt(D)) + row-sum via ACT accum_out
           attn tiles transposed on PE, attnT @ V accumulated in PSUM
           out = attn_out * (1/(sum+eps)) + x  fused on DVE, DMA out.

Matmuls run in float32r (tf32-like single-pass mode, ~4x faster than fp32
on TRN2's PE; measured ~1.5e-4 relative error per matmul).
"""

import numpy as np
from contextlib import ExitStack

import concourse.bacc as bacc
import concourse.tile as tile
import concourse.mybir as mybir
from concourse.bass_utils import run_bass_kernel_spmd
from concourse.masks import make_identity

# problem dims (hardcoded per harness contract)
B, S, D = 16, 2048, 1024
NCORES, P = 8, 128
BPC = B // NCORES          # 2 batches per core
NFC = D // P               # 8 feature chunks of 128
NDC = D // P               # 8 contraction chunks of 128
NKT = S // P               # 16 key tiles of 128
W5 = 512
NST = S // W5              # 4 strips of 512
NDH = D // W5              # 2 output-feature halves of 512
SM_SCALE = float(1.0 / np.sqrt(D))
EPS = 1e-6

F32 = mybir.dt.float32
F32R = mybir.dt.float32r
MM_DT = F32R               # matmul operand dtype: F32R (fast) or F32 (exact)

AF = mybir.ActivationFunctionType
ALU = mybir.AluOpType
AX = mybir.AxisListType


def _r(ap):
    """View a DRAM fp32 AP in the matmul dtype (byte-identical)."""
    return ap.bitcast(MM_DT) if MM_DT is not F32 else ap


def _build():
    nc = bacc.Bacc("TRN2", target_bir_lowering=False, debug=False)

    xT = nc.dram_tensor("xT", [BPC, D, S], F32, kind="ExternalInput").ap()
    yT = nc.dram_tensor("yT", [BPC, D, S], F32, kind="ExternalInput").ap()
    xr = nc.dram_tensor("xr", [BPC, S, D], F32, kind="ExternalInput").ap()
    Wq = nc.dram_tensor("Wq", [D, D], F32, kind="ExternalInput").ap()
    Wk = nc.dram_tensor("Wk", [D, D], F32, kind="ExternalInput").ap()
    Wv = nc.dram_tensor("Wv", [D, D], F32, kind="ExternalInput").ap()
    bq = nc.dram_tensor("bq", [D], F32, kind="ExternalInput").ap()
    bk = nc.dram_tensor("bk", [D], F32, kind="ExternalInput").ap()
    bv = nc.dram_tensor("bv", [D], F32, kind="ExternalInput").ap()
    out = nc.dram_tensor("out", [BPC, S, D], F32, kind="ExternalOutput").ap()

    with tile.TileContext(nc) as tc, ExitStack() as ctx:
        const = ctx.enter_context(tc.tile_pool(name="const", bufs=1))
        kvp = ctx.enter_context(tc.tile_pool(name="kvp", bufs=1))
        psum = ctx.enter_context(tc.tile_pool(name="psum", bufs=6, space="PSUM"))
        dram = ctx.enter_context(tc.tile_pool(name="dram", bufs=2, space="DRAM"))

        # ---- constants
        ident = const.tile([P, P], F32)
        make_identity(nc, ident)
        ones1f = const.tile([1, P], F32)
        nc.vector.memset(ones1f, 1.0)
        ones1 = const.tile([1, P], MM_DT)
        nc.vector.tensor_copy(ones1, ones1f)
        bqs = const.tile([P, NFC], F32)
        nc.sync.dma_start(out=bqs, in_=bq.rearrange("(fc p) -> p fc", p=P))
        bks = const.tile([P, NFC], F32)
        nc.sync.dma_start(out=bks, in_=bk.rearrange("(fc p) -> p fc", p=P))
        bvs = const.tile([1, D], MM_DT)
        nc.sync.dma_start(out=bvs, in_=_r(bv.rearrange("(a d) -> a d", a=1)))

        for b in range(BPC):
            # resident K^T [f-major] and V [seq-major] for this batch
            KT = kvp.tile([P, NFC, S], MM_DT, tag="KT")
            V = kvp.tile([P, NKT, D], MM_DT, tag="V")
            qspill = dram.tile([D, S], MM_DT, tag="qspill")
            qview = qspill.rearrange("(fc p) s -> p fc s", p=P)

            # ================= stage A: projections =================
            with tc.tile_pool(name=f"stA_{b}", bufs=1) as ap_, \
                 tc.tile_pool(name=f"strips_{b}", bufs=10) as strips:

                # --- phase Q: QT = Wq^T @ xT (+bq), f-major, spill to DRAM
                wsb = ap_.tile([P, NDC, D], MM_DT, tag="w")
                for dc in range(NDC):
                    nc.sync.dma_start(out=wsb[:, dc, :], in_=_r(Wq[dc * P:(dc + 1) * P, :]))
                for st in range(NST):
                    xq = []
                    for dc in range(NDC):
                        t = strips.tile([P, W5], MM_DT, tag="strip", name=f"xq{dc}")
                        nc.sync.dma_start(out=t, in_=_r(xT[b, dc * P:(dc + 1) * P, st * W5:(st + 1) * W5]))
                        xq.append(t)
                    qts = ap_.tile([P, NFC, W5], MM_DT, tag="qspill_sb")
                    for fc in range(NFC):
                        ps = psum.tile([P, W5], F32, tag="mm512", name="psq")
                        for dc in range(NDC):
                            nc.tensor.matmul(ps, wsb[:, dc, fc * P:(fc + 1) * P], xq[dc],
                                             start=(dc == 0), stop=(dc == NDC - 1))
                        nc.scalar.activation(qts[:, fc, :], ps, AF.Identity, bias=bqs[:, fc:fc + 1])
                    nc.sync.dma_start(out=qview[:, :, st * W5:(st + 1) * W5], in_=qts)

                # --- phase K: KT = Wk^T @ yT (+bk), f-major, SBUF resident
                wsb = ap_.tile([P, NDC, D], MM_DT, tag="w")
                for dc in range(NDC):
                    nc.sync.dma_start(out=wsb[:, dc, :], in_=_r(Wk[dc * P:(dc + 1) * P, :]))
                for st in range(NST):
                    yq = []
                    for dc in range(NDC):
                        t = strips.tile([P, W5], MM_DT, tag="strip", name=f"yq{dc}")
                        nc.sync.dma_start(out=t, in_=_r(yT[b, dc * P:(dc + 1) * P, st * W5:(st + 1) * W5]))
                        yq.append(t)
                    for fc in range(NFC):
                        ps = psum.tile([P, W5], F32, tag="mm512", name="psk")
                        for dc in range(NDC):
                            nc.tensor.matmul(ps, wsb[:, dc, fc * P:(fc + 1) * P], yq[dc],
                                             start=(dc == 0), stop=(dc == NDC - 1))
                        nc.scalar.activation(KT[:, fc, st * W5:(st + 1) * W5], ps, AF.Identity,
                                             bias=bks[:, fc:fc + 1])

                # --- phase V: V = y @ Wv (+bv), seq-major, SBUF resident
                wsb = ap_.tile([P, NDC, D], MM_DT, tag="w")
                for dc in range(NDC):
                    nc.sync.dma_start(out=wsb[:, dc, :], in_=_r(Wv[dc * P:(dc + 1) * P, :]))
                for st in range(NST):
                    yq = []
                    for dc in range(NDC):
                        t = strips.tile([P, W5], MM_DT, tag="strip", name=f"yv{dc}")
                        nc.sync.dma_start(out=t, in_=_r(yT[b, dc * P:(dc + 1) * P, st * W5:(st + 1) * W5]))
                        yq.append(t)
                    for ks in range(NST):
                        kt = st * NST + ks
                        for dh in range(NDH):
                            ps = psum.tile([P, W5], F32, tag="mm512", name="psv")
                            for dc in range(NDC):
                                nc.tensor.matmul(ps, yq[dc][:, ks * P:(ks + 1) * P],
                                                 wsb[:, dc, dh * W5:(dh + 1) * W5],
                                                 start=(dc == 0), stop=False)
                            nc.tensor.matmul(ps, ones1, bvs[:, dh * W5:(dh + 1) * W5],
                                             start=False, stop=True)
                            nc.vector.tensor_copy(V[:, kt, dh * W5:(dh + 1) * W5], ps)

            # ================= stage B: attention =================
            with tc.tile_pool(name=f"stB_{b}", bufs=2) as bp, \
                 tc.tile_pool(name=f"attp_{b}", bufs=4) as attp:
                for st in range(NST):
                    qts = bp.tile([P, NFC, W5], MM_DT, tag="qts")
                    nc.sync.dma_start(out=qts, in_=qview[:, :, st * W5:(st + 1) * W5])
                    for qq in range(4):
                        qt = st * 4 + qq
                        # logits in 4 PSUM banks
                        lg = []
                        for kc in range(NST):
                            ps = psum.tile([P, W5], F32, tag="mm512", name=f"lg{kc}")
                            for fc in range(NFC):
                                nc.tensor.matmul(ps, qts[:, fc, qq * P:(qq + 1) * P],
                                                 KT[:, fc, kc * W5:(kc + 1) * W5],
                                                 start=(fc == 0), stop=(fc == NFC - 1))
                            lg.append(ps)
                        # softmax stats
                        m4 = bp.tile([P, NST], F32, tag="m4")
                        for kc in range(NST):
                            nc.vector.reduce_max(m4[:, kc:kc + 1], lg[kc], axis=AX.X)
                        mx = bp.tile([P, 1], F32, tag="mx")
                        nc.vector.reduce_max(mx, m4, axis=AX.X)
                        nmb = bp.tile([P, 1], F32, tag="nmb")
                        nc.scalar.mul(nmb, mx, -SM_SCALE)
                        ex = bp.tile([P, S], F32, tag="ex")
                        z4 = bp.tile([P, NST], F32, tag="z4")
                        for kc in range(NST):
                            nc.scalar.activation(ex[:, kc * W5:(kc + 1) * W5], lg[kc], AF.Exp,
                                                 bias=nmb, scale=SM_SCALE,
                                                 accum_out=z4[:, kc:kc + 1])
                        zs = bp.tile([P, 1], F32, tag="zs")
                        nc.vector.reduce_sum(zs, z4, axis=AX.X)
                        z2 = bp.tile([P, 1], F32, tag="z2")
                        nc.vector.tensor_scalar_add(z2, zs, EPS)
                        rz = bp.tile([P, 1], F32, tag="rz")
                        nc.vector.reciprocal(rz, z2)
                        # attn^T tiles (PE transpose) and attn @ V
                        xrs = bp.tile([P, D], F32, tag="xrs")
                        nc.sync.dma_start(out=xrs, in_=xr[b, qt * P:(qt + 1) * P, :])
                        ao = [psum.tile([P, W5], F32, tag="mm512", name=f"ao{dh}")
                              for dh in range(NDH)]
                        for kt2 in range(NKT):
                            tp = psum.tile([P, P], F32, tag="tp", bufs=2)
                            nc.tensor.transpose(tp, ex[:, kt2 * P:(kt2 + 1) * P], ident)
                            at = attp.tile([P, P], MM_DT, tag="att")
                            nc.vector.tensor_copy(at, tp)
                            for dh in range(NDH):
                                nc.tensor.matmul(ao[dh], at, V[:, kt2, dh * W5:(dh + 1) * W5],
                                                 start=(kt2 == 0), stop=(kt2 == NKT - 1))
                        osb = bp.tile([P, D], F32, tag="osb")
                        for dh in range(NDH):
                            nc.vector.scalar_tensor_tensor(
                                osb[:, dh * W5:(dh + 1) * W5], ao[dh], rz,
                                xrs[:, dh * W5:(dh + 1) * W5],
                                op0=ALU.mult, op1=ALU.add)
                        nc.sync.dma_start(out=out[b, qt * P:(qt + 1) * P, :], in_=osb)

    nc.compile()
    return nc


_NC_CACHE = {}


def _get_nc():
    if "nc" not in _NC_CACHE:
        _NC_CACHE["nc"] = _build()
    return _NC_CACHE["nc"]


def _make_in_maps(x, y, Wq, bq, Wk, bk, Wv, bv):
    x = np.asarray(x, dtype=np.float32)
    y = np.asarray(y, dtype=np.float32)
    xT = np.ascontiguousarray(x.transpose(0, 2, 1))
    yT = np.ascontiguousarray(y.transpose(0, 2, 1))
    Wq = np.ascontiguousarray(np.asarray(Wq, dtype=np.float32))
    Wk = np.ascontiguousarray(np.asarray(Wk, dtype=np.float32))
    Wv = np.ascontiguousarray(np.asarray(Wv, dtype=np.float32))
    bq = np.ascontiguousarray(np.asarray(bq, dtype=np.float32))
    bk = np.ascontiguousarray(np.asarray(bk, dtype=np.float32))
    bv = np.ascontiguousarray(np.asarray(bv, dtype=np.float32))
    in_maps = []
    for c in range(NCORES):
        sl = slice(c * BPC, (c + 1) * BPC)
        in_maps.append({
            "xT": np.ascontiguousarray(xT[sl]),
            "yT": np.ascontiguousarray(yT[sl]),
            "xr": np.ascontiguousarray(x[sl]),
            "Wq": Wq, "Wk": Wk, "Wv": Wv,
            "bq": bq, "bk": bk, "bv": bv,
        })
    return in_maps


def kernel(x, y, Wq, bq, Wk, bk, Wv, bv):
    nc = _get_nc()
    in_maps = _make_in_maps(x, y, Wq, bq, Wk, bk, Wv, bv)
    res = run_bass_kernel_spmd(nc, in_maps, core_ids=list(range(NCORES)))
    return np.concatenate([r["out"] for r in res.results], axis=0)
